# revision 9
# baseline (speedup 1.0000x reference)
"""TRN2 Bass kernel for nn_MetaHyperNetwork_20830591385783 (moe_routing).

Reference computation:
  sim  = (hw @ hw_emb.T) / sqrt(10)            # [50]
  gate = softmax(sin(sim))                     # [50]
  idx  = round(x[0,0] * 100)                   # scalar int in [0,100]
  rows = expert_emb[:, idx, :]                 # [50, 30]
  out  = einsum('e,ed->d', gate, rows).reshape(6, 5)

Distribution strategy (8 NeuronCores): the gate inputs (hw, hw_emb, x) are
tiny and replicated; the expert table is sharded over the interval axis
(13 intervals per core). Each core computes the full gate, selects its
local rows with a register-driven dynamic SBUF slice, and writes the
(normalized) output only if it owns idx (conditional DMA; other cores'
outputs stay zero). The host unshards by summing the 8 per-core outputs.

Per-core device program (raw Bass, manual semaphores):
  - TWO packed input DMAs: a small "gate pack" (host-transposed hw_emb.T,
    hw, x, lo*30) that unblocks the gate chain early, and the [50, W*30]
    expert-table chunk.
  - PE matmul: sim[50,1] = hw_embT^T @ hw.
  - ACT: sin(scale=1/sqrt(10)) then tanh(scale=0.5); DVE computes
    exp(s) = (1+t)/(1-t) = 2/(1-t) - 1, so the whole kernel needs only
    one ACT table set (warmed by a dummy Sin at program start; sin and
    exp live in different table sets, and a set switch costs ~2.7us).
  - DVE: idx = round(x*100) (the HW f32->i32 output conversion is
    round-to-nearest-even, bit-exact with jnp.round; CoreSim diverges --
    it truncates -- HW is truth). GPSIMD loads idx into registers,
    computes clamp(idx*30-lo30, 0, E-30), and copies the [50,30] rows
    (dynamic SBUF offset) into rows[:,0:30] with col 30 pre-set to 1.0
    (gives the softmax normalizer Z for free).
  - PE matmul: [1,31] = w^T @ rows_ones; DVE: out = o[0:30] * (1/Z).
  - Sync: conditional output DMA (ownership test on SP registers).
"""

import math
import sys
from contextlib import ExitStack

import numpy as np

for _p in ("/opt/trn_rl_repo", "/root/.axon_site/_ro/trn_rl_repo"):
    if _p not in sys.path:
        sys.path.append(_p)

import concourse.bass as bass
import concourse.mybir as mybir

FP32 = mybir.dt.float32
I32 = mybir.dt.int32
AF = mybir.ActivationFunctionType
ALU = mybir.AluOpType

NE = 50           # experts
NI = 101          # intervals
DD = 30           # expert embedding dim
DH = 10           # hw embed dim
RSQRT_DH = 1.0 / math.sqrt(DH)
N_CORES = 8
W_SHARD = 13      # ceil(101/8) intervals per core


G_C = 56  # gate-pack cols: heT [10,50] | hw col 50 | x [0,51] | lo30 [0,52]


def _pack_cols(W):
    E = W * DD
    return E


class _NoBarrier:
    """Suppress Bass.all_engine_barrier while constructing the Bass object
    (we replace the one cross-engine preamble dependency with our own sem)."""

    def __enter__(self):
        self._orig = bass.Bass.all_engine_barrier
        bass.Bass.all_engine_barrier = lambda self_, *a, **k: None
        return self

    def __exit__(self, *exc):
        bass.Bass.all_engine_barrier = self._orig


def _finish_block(nc, blk):
    """Close an engine block WITHOUT the all-engine exit barrier (same
    branch/switch bookkeeping as BassBlock.__exit__): Sync explicitly waits
    for the output DMA; every other engine simply halts at program end."""
    for engine, last_body in blk.last_body.items():
        with nc.body(last_body, parent=nc.cur_bb, allow_existing_parent=True):
            engine.br(blk.end_bb)
    nc.switch_bb(blk.end_bb)
    nc.cur_block = None


def build_nc(sharded: bool = True, wait_out: bool = True):
    W = W_SHARD if sharded else NI
    E = _pack_cols(W)

    with _NoBarrier():
        nc = bass.Bass(
            "TRN2", target_bir_lowering=False, debug=False, monotonic_sem_count=0
        )

    gate_d = nc.dram_tensor("gate_pack", [DH, G_C], FP32, kind="ExternalInput")
    pack_d = nc.dram_tensor("pack", [NE, E], FP32, kind="ExternalInput")
    out_d = nc.dram_tensor("out", [1, DD], FP32, kind="ExternalOutput")

    if True:
        with ExitStack() as ctx:
            e = ctx.enter_context
            P_sb = e(nc.sbuf_tensor("P_sb", [NE, E], FP32))
            G_sb = e(nc.sbuf_tensor("G_sb", [DH, G_C], FP32))
            scr = e(nc.sbuf_tensor("scr", [1, 1], FP32))
            z2 = e(nc.sbuf_tensor("z2", [1, 1], FP32))
            idx_t = e(nc.sbuf_tensor("idx_t", [1, 1], I32))
            s_sb = e(nc.sbuf_tensor("s_sb", [NE, 1], FP32))
            th_sb = e(nc.sbuf_tensor("th_sb", [NE, 1], FP32))
            num_sb = e(nc.sbuf_tensor("num_sb", [NE, 1], FP32))
            den_sb = e(nc.sbuf_tensor("den_sb", [NE, 1], FP32))
            rd_sb = e(nc.sbuf_tensor("rd_sb", [NE, 1], FP32))
            w_sb = e(nc.sbuf_tensor("w_sb", [NE, 1], FP32))
            rows_sb = e(nc.sbuf_tensor("rows_sb", [NE, DD + 1], FP32))
            r_sb = e(nc.sbuf_tensor("r_sb", [1, 1], FP32))
            o_sb = e(nc.sbuf_tensor("o_sb", [1, DD], FP32))

            sim_ps = e(nc.psum_tensor("sim_ps", [NE, 1], FP32))
            o_ps = e(nc.psum_tensor("o_ps", [1, DD + 1], FP32))

            sem_in = e(nc.semaphore("sem_in"))
            sem_out = e(nc.semaphore("sem_out"))
            sem_pe = e(nc.semaphore("sem_pe"))
            sem_dve = e(nc.semaphore("sem_dve"))
            sem_act = e(nc.semaphore("sem_act"))
            sem_gp = e(nc.semaphore("sem_gp"))
            sem_res = e(nc.semaphore("sem_res"))
            sem_c = e(nc.semaphore("sem_c"))
            sem_g = e(nc.semaphore("sem_g"))

            W_COUNT = 4  # sem_dve value once w_sb is written

            x_ap = G_sb[0:1, NE + 1:NE + 2]
            lo30_ap = G_sb[0:1, NE + 2:NE + 3].bitcast(I32)
            heT_ap = G_sb[0:DH, 0:NE]
            hw_ap = G_sb[0:DH, NE:NE + 1]

            block = bass.BassBlock(nc, f"block_{nc.next_id()}")
            nc.cur_block = block

            @block.sync
            def _(sync):
                sync.dma_start(G_sb[:], gate_d.ap()).then_inc(sem_g, 16)
                sync.dma_start(P_sb[:], pack_d.ap()).then_inc(sem_in, 16)
                if sharded:
                    # ownership: 0 <= idx*30 - lo30 <= E-30
                    sync.wait_ge(sem_dve, 1)
                    r1 = nc.alloc_register(mybir.EngineType.SP, "sy_idx")
                    r2 = nc.alloc_register(mybir.EngineType.SP, "sy_lo")
                    ra = nc.alloc_register(mybir.EngineType.SP, "sy_a")
                    rb = nc.alloc_register(mybir.EngineType.SP, "sy_b")
                    sync.reg_load(r1, idx_t[0:1, 0:1])
                    sync.reg_load(r2, lo30_ap)
                    sync.reg_alu(r1, r1, DD, ALU.mult)
                    sync.reg_alu(r1, r1, r2, ALU.subtract)
                    sync.reg_alu(ra, r1, 0, ALU.is_ge)
                    sync.reg_alu(rb, r1, E - DD, ALU.is_le)
                    sync.reg_alu(ra, ra, rb, ALU.bitwise_and)
                    own = sync.snap(ra, min_val=0, max_val=1)
                    # wait attached to the DMA itself: the cond-AP ALU work
                    # runs while Sync is otherwise idle, before the result
                    sync.dma_start(
                        out_d.ap(), o_sb[:], cond=own, single_packet=True
                    )._wait_ge(sem_res, 1).then_inc(sem_out, 16)
                else:
                    sync.wait_ge(sem_res, 1)
                    sync.dma_start(out_d.ap(), o_sb[:], single_packet=True).then_inc(
                        sem_out, 16
                    )
                if wait_out:
                    sync.wait_ge(sem_out, 16)

            @block.scalar
            def _(act):
                # warm the (single) ACT table set before data arrives
                act.wait_ge(sem_c, 1)
                act.activation(scr[:], z2[0:1, 0:1], AF.Sin)
                act.wait_ge(sem_pe, 1)
                act.activation(s_sb[:], sim_ps[:], AF.Sin, scale=RSQRT_DH).then_inc(sem_act, 1)
                act.wait_ge(sem_act, 1)
                act.activation(th_sb[:], s_sb[:], AF.Tanh, scale=0.5).then_inc(sem_act, 1)

            @block.tensor
            def _(pe):
                pe.wait_ge(sem_g, 16)
                pe.matmul(sim_ps[:], heT_ap, hw_ap, start=True, stop=True).then_inc(sem_pe, 1)
                pe.wait_ge(sem_dve, W_COUNT)
                pe.wait_ge(sem_gp, 1)
                pe.matmul(o_ps[:], w_sb[:], rows_sb[:], start=True, stop=True).then_inc(sem_pe, 2)

            @block.gpsimd
            def _(gp):
                gp.enable_hardware_checks = False
                gp.memset(z2[:], 0.0).then_inc(sem_c, 1)
                gp.memset(rows_sb[:, DD:DD + 1], 1.0)
                gp.wait_ge(sem_g, 16)
                if sharded:
                    r2 = nc.alloc_register(mybir.EngineType.Pool, "lo_reg")
                    gp.reg_load(r2, lo30_ap)
                gp.wait_ge(sem_dve, 1)
                r1 = nc.alloc_register(mybir.EngineType.Pool, "idx_reg")
                gp.reg_load(r1, idx_t[0:1, 0:1])
                gp.reg_alu(r1, r1, DD, ALU.mult)
                if sharded:
                    gp.reg_alu(r1, r1, r2, ALU.subtract)
                gp.reg_alu(r1, r1, 0, ALU.max)
                gp.reg_alu(r1, r1, E - DD, ALU.min)
                off = gp.snap(r1, min_val=0, max_val=E - DD)
                gp.wait_ge(sem_in, 16)
                gp.tensor_copy(rows_sb[:, 0:DD], P_sb[:, bass.ds(off, DD)]).then_inc(sem_gp, 1)

            @block.vector
            def _(dve):
                dve.wait_ge(sem_g, 16)
                # idx = round(x*100): the HW f32->i32 output conversion rounds
                # to nearest-even, exactly matching jnp.round. (CoreSim
                # truncates instead -- the simulator diverges here; HW is truth.)
                dve.tensor_scalar(idx_t[:], x_ap, 100.0, None, ALU.mult).then_inc(sem_dve, 1)
                # w = exp(sin(sim)) = (1+t)/(1-t) = 2/(1-t) - 1, t = tanh(sin/2)
                dve.wait_ge(sem_act, 2)
                dve.tensor_scalar(den_sb[:], th_sb[:], -1.0, 1.0, ALU.mult, ALU.add).then_inc(sem_dve, 1)
                dve.wait_ge(sem_dve, 2)
                dve.reciprocal(rd_sb[:], den_sb[:]).then_inc(sem_dve, 1)
                dve.wait_ge(sem_dve, 3)
                dve.tensor_scalar(w_sb[:], rd_sb[:], 2.0, -1.0, ALU.mult, ALU.add).then_inc(sem_dve, 1)
                # out = o_ps[0:30] / Z   (Z = o_ps[30], via the ones column)
                dve.wait_ge(sem_pe, 3)
                dve.reciprocal(r_sb[:], o_ps[0:1, DD:DD + 1]).then_inc(sem_dve, 1)
                dve.wait_ge(sem_dve, 5)
                dve.tensor_scalar(
                    o_sb[:], o_ps[0:1, 0:DD], r_sb[0:1, 0:1], None, ALU.mult
                ).then_inc(sem_res, 1)

            _finish_block(nc, block)

    return nc


def make_packs(x, hw, hw_emb, expert_emb, sharded: bool = True):
    """Host-side input staging: slice/reshape/transpose the inputs into one
    packed [50, C] array per core (plus the compile-time shard constant
    lo*30 as int32 bits). No data-dependent computation happens here."""
    x = np.ascontiguousarray(x, dtype=np.float32)
    hw = np.ascontiguousarray(hw, dtype=np.float32)
    he = np.ascontiguousarray(hw_emb, dtype=np.float32)
    ex = np.ascontiguousarray(expert_emb, dtype=np.float32).reshape(NE, NI, DD)

    W = W_SHARD if sharded else NI
    E = _pack_cols(W)

    packs = []
    n = N_CORES if sharded else 1
    for c in range(n):
        p = np.zeros((NE, E), dtype=np.float32)
        if sharded:
            lo = W * c
            hi = min(NI, lo + W)
            p[:, 0:(hi - lo) * DD] = ex[:, lo:hi, :].reshape(NE, -1)
        else:
            lo = 0
            p[:, 0:E] = ex.reshape(NE, -1)
        g = np.zeros((DH, G_C), dtype=np.float32)
        g[:, 0:NE] = he.T
        g[:, NE:NE + 1] = hw.reshape(DH, 1)
        g[0, NE + 1] = x.reshape(-1)[0]
        g[0, NE + 2] = np.array(lo * DD, dtype=np.int32).view(np.float32)
        packs.append({"pack": p, "gate_pack": g})
    if not sharded:
        packs = packs * N_CORES
    return packs


_NC_CACHE = {}


def _get_nc(sharded: bool = True):
    if sharded not in _NC_CACHE:
        _NC_CACHE[sharded] = build_nc(sharded)
    return _NC_CACHE[sharded]


def kernel(x, hw, hw_emb, expert_emb):
    from concourse.bass_utils import run_bass_kernel_spmd

    nc = _get_nc(sharded=True)
    packs = make_packs(x, hw, hw_emb, expert_emb, sharded=True)
    res = run_bass_kernel_spmd(nc, packs, list(range(N_CORES)))
    # unshard: exactly one core (the idx owner) wrote its output; the other
    # cores' outputs are all-zero, so the sum is the full result.
    out = np.sum([res.results[c]["out"] for c in range(N_CORES)], axis=0)
    return out.reshape(6, 5).astype(np.float32)
